# revision 1
# baseline (speedup 1.0000x reference)
"""Distributed Trainium2 (Bass/Tile) kernel for the KPCL contrastive loss.

Math (matches the jax reference):
  x1 = f + sign(f) * normalize(n1, 1e-8) * 0.1
  x2 = x1 + sign(x1) * normalize(n2, 1e-8) * 0.1
     = sign(f) * (|f| + 0.1*n1/max(||n1||,eps) + 0.1*n2/max(||n2||,eps))
  p  = relu(x2 @ W1 + b1) @ W2 + b2
  z  = p / max(||p||, 1e-6)
  sim = z @ z_all.T / T ;  lse_i = log(sum_j exp(sim_ij)) ; pos_i = sim_ii
  loss = mean(-pos + lse) + log(2)

Sharding: rows (N=8192) split across 8 cores, 1024 rows each. Each core
computes its z block in transposed layout zT [128, 1024], AllGathers zT
to [1024, 1024] (8 rank blocks of [128, 1024] = z_all^T), then computes
its row-block of sim as 128x512 matmuls (K=128 contraction) with a fused
exp+row-sum on the scalar engine. Per-core output is the scalar
sum_i(log(sumexp_i) - pos_i); the host sums, divides by N and adds log2.
"""

import sys

for _p in ("/opt/trn_rl_repo",):
    if _p not in sys.path:
        sys.path.append(_p)

import numpy as np

import concourse.bass as bass
import concourse.tile as tile
from concourse import mybir
from concourse.bass_utils import run_bass_kernel_spmd
from concourse.masks import make_identity

F32 = mybir.dt.float32
BF16 = mybir.dt.bfloat16
U32 = mybir.dt.uint32

N_CORES = 8
N = 8192
ROWS = N // N_CORES          # 1024 rows per core
D_IN = 512
D_PROJ = 128
TEMP = 0.15
P = 128                      # partitions
NBLK = ROWS // P             # 8 row-blocks per core
INV_T = 1.0 / TEMP

AF = mybir.ActivationFunctionType
OP = mybir.AluOpType


def split_excess_waits(nc: bass.Bass, max_waits: int = 1) -> int:
    """Hoist excess sem waits onto same-engine nop carriers.

    The walrus build in this image rejects instructions carrying more
    than ~2 sync commands ("Too many sync wait commands"), but Tile's
    wait assignment freely emits 2-3 waits per instruction. Splitting
    the waits onto preceding nop instructions on the same engine queue
    is semantically identical (engine program order is preserved).
    """
    nmoved = 0
    for f in nc.m.functions:
        for b in f.blocks:
            il = b.instructions
            i = 0
            while i < len(il):
                inst = il[i]
                si = inst.sync_info
                if si is None or not si.on_wait or len(si.on_wait) <= max_waits:
                    i += 1
                    continue
                eng = inst.engine
                if eng is None:
                    i += 1
                    continue
                waits = list(si.on_wait)
                keep = waits[-max_waits:]
                excess = waits[:-max_waits]
                carriers = []
                for w in excess:
                    nop = nc.engines[eng].nop().ins
                    for f2 in nc.m.functions:
                        for b2 in f2.blocks:
                            try:
                                b2.instructions.remove(nop)
                            except ValueError:
                                pass
                    nop.sync_info = mybir.SyncInfo(on_wait=[w], on_update=[])
                    carriers.append(nop)
                inst.sync_info = mybir.SyncInfo(on_wait=keep,
                                                on_update=list(si.on_update))
                for c in reversed(carriers):
                    il.insert(i, c)
                i += 1 + len(carriers)
                nmoved += len(excess)
    return nmoved


def build_nc(phase: str = "full") -> bass.Bass:
    # phase: "A" (local z only), "AG" (+allgather+loads), "full"
    nc = bass.Bass("TRN2", target_bir_lowering=False, debug=False,
                   num_devices=N_CORES)

    f_d = nc.dram_tensor("features", [ROWS, D_IN], F32, kind="ExternalInput")
    u1_d = nc.dram_tensor("noise1", [ROWS, D_IN], F32, kind="ExternalInput")
    u2_d = nc.dram_tensor("noise2", [ROWS, D_IN], F32, kind="ExternalInput")
    w1_d = nc.dram_tensor("W1", [D_IN, D_PROJ], F32, kind="ExternalInput")
    b1_d = nc.dram_tensor("b1", [D_PROJ, 1], F32, kind="ExternalInput")
    w2_d = nc.dram_tensor("W2", [D_PROJ, D_PROJ], F32, kind="ExternalInput")
    b2_d = nc.dram_tensor("b2", [D_PROJ, 1], F32, kind="ExternalInput")
    out_d = nc.dram_tensor("out", [1, 1], F32, kind="ExternalOutput")

    # collective bounce buffers (internal DRAM; AG output must be Shared)
    zT_bounce = nc.dram_tensor("zT_bounce", [P, ROWS], F32)
    zall_bounce = nc.dram_tensor("zall_bounce", [N_CORES * P, ROWS], F32,
                                 addr_space="Shared")

    with tile.TileContext(nc) as tc:
        with (
            tc.tile_pool(name="singles", bufs=1) as singles,
            tc.tile_pool(name="work", bufs=3) as work,
            tc.tile_pool(name="small", bufs=3) as small,
            tc.tile_pool(name="expsc", bufs=2) as expsc,
        ):
            # ---- constants / persistent tiles ----
            w1t = singles.tile([P, 4, P], F32)      # W1 k-chunks (lhsT)
            for c in range(4):
                nc.sync.dma_start(w1t[:, c, :], w1_d[c * P:(c + 1) * P, :])
            w2t = singles.tile([P, P], F32)
            nc.sync.dma_start(w2t[:], w2_d[:, :])
            b1t = singles.tile([P, 1], F32)
            nc.sync.dma_start(b1t[:], b1_d[:, :])
            b2t = singles.tile([P, 1], F32)
            nc.sync.dma_start(b2t[:], b2_d[:, :])

            ident = singles.tile([P, P], F32)
            make_identity(nc, ident[:])
            ones_col = singles.tile([P, 1], F32)
            nc.gpsimd.memset(ones_col[:], 1.0)
            ones_row = singles.tile([1, P], F32)
            nc.gpsimd.memset(ones_row[:], 1.0)
            zbias = singles.tile([P, 1], F32)
            nc.gpsimd.memset(zbias[:], 0.0)
            zbias1 = singles.tile([1, 1], F32)
            nc.gpsimd.memset(zbias1[:], 0.0)

            zT = singles.tile([P, ROWS], F32)       # z^T for this core
            logS = singles.tile([P, NBLK], F32)     # log(sumexp) per block
            pos_all = singles.tile([1, ROWS], F32)  # diag(sim) per local row
            zallT = singles.tile([P, N_CORES, ROWS], F32)  # gathered z_all^T

            # =========== Phase A: augment + projection + normalize ==========
            with (
                tc.tile_pool(name="psA2", bufs=2, space="PSUM") as psA2,
                tc.tile_pool(name="psA1", bufs=1, space="PSUM") as psA1,
            ):
                for m in range(NBLK):
                    rs = slice(m * P, (m + 1) * P)
                    ft = work.tile([P, D_IN], F32, tag="F")
                    nc.sync.dma_start(ft[:], f_d[rs, :])
                    u1 = work.tile([P, D_IN], F32, tag="U1")
                    nc.sync.dma_start(u1[:], u1_d[rs, :])
                    u2 = work.tile([P, D_IN], F32, tag="U2")
                    nc.sync.dma_start(u2[:], u2_d[rs, :])

                    # noise norms: s = sum(u^2); r = 0.1/max(sqrt(s), 1e-8)
                    sq = work.tile([P, D_IN], F32, tag="sq")
                    s1 = small.tile([P, 1], F32, tag="s1")
                    nc.vector.scalar_tensor_tensor(
                        out=sq[:], in0=u1[:], scalar=1.0, in1=u1[:],
                        op0=OP.mult, op1=OP.mult, accum_out=s1[:])
                    sq2 = work.tile([P, D_IN], F32, tag="sq")
                    s2 = small.tile([P, 1], F32, tag="s2")
                    nc.vector.scalar_tensor_tensor(
                        out=sq2[:], in0=u2[:], scalar=1.0, in1=u2[:],
                        op0=OP.mult, op1=OP.mult, accum_out=s2[:])

                    n1 = small.tile([P, 1], F32, tag="n1")
                    nc.scalar.activation(n1[:], s1[:], AF.Sqrt, bias=zbias[:])
                    n2 = small.tile([P, 1], F32, tag="n2")
                    nc.scalar.activation(n2[:], s2[:], AF.Sqrt, bias=zbias[:])
                    # rN = 1 / (10 * max(n, 1e-8))  == 0.1 / max(n, 1e-8)
                    n1c = small.tile([P, 1], F32, tag="n1c")
                    nc.vector.tensor_scalar(out=n1c[:], in0=n1[:], scalar1=1e-8,
                                            scalar2=10.0, op0=OP.max, op1=OP.mult)
                    r1 = small.tile([P, 1], F32, tag="r1")
                    nc.vector.reciprocal(r1[:], n1c[:])
                    n2c = small.tile([P, 1], F32, tag="n2c")
                    nc.vector.tensor_scalar(out=n2c[:], in0=n2[:], scalar1=1e-8,
                                            scalar2=10.0, op0=OP.max, op1=OP.mult)
                    r2 = small.tile([P, 1], F32, tag="r2")
                    nc.vector.reciprocal(r2[:], n2c[:])

                    # |f| and sign bit
                    absf = work.tile([P, D_IN], F32, tag="absf")
                    nc.vector.tensor_scalar(
                        out=absf[:].bitcast(U32), in0=ft[:].bitcast(U32),
                        scalar1=0x7FFFFFFF, scalar2=None, op0=OP.bitwise_and)
                    sgn = work.tile([P, D_IN], F32, tag="sgn")
                    nc.vector.tensor_scalar(
                        out=sgn[:].bitcast(U32), in0=ft[:].bitcast(U32),
                        scalar1=0x80000000, scalar2=None, op0=OP.bitwise_and)

                    # a = |f| + u1*r1 + u2*r2 ; x2 = a | signbit
                    bt = work.tile([P, D_IN], F32, tag="bt")
                    nc.vector.scalar_tensor_tensor(
                        out=bt[:], in0=u1[:], scalar=r1[:], in1=absf[:],
                        op0=OP.mult, op1=OP.add)
                    at = work.tile([P, D_IN], F32, tag="at")
                    nc.vector.scalar_tensor_tensor(
                        out=at[:], in0=u2[:], scalar=r2[:], in1=bt[:],
                        op0=OP.mult, op1=OP.add)
                    x2 = work.tile([P, D_IN], F32, tag="x2")
                    nc.vector.tensor_tensor(
                        out=x2[:].bitcast(U32), in0=at[:].bitcast(U32),
                        in1=sgn[:].bitcast(U32), op=OP.bitwise_or)

                    # transpose x2 into [512part-chunks, 128rows]
                    xT = work.tile([P, 4, P], F32, tag="xT")
                    for c in range(4):
                        tp = psA2.tile([P, P], F32, tag="tp")
                        nc.tensor.transpose(tp[:], x2[:, c * P:(c + 1) * P],
                                            ident[:])
                        nc.any.tensor_copy(xT[:, c, :], tp[:])

                    # hT = relu(W1^T-chunks contraction + b1)
                    hps = psA2.tile([P, P], F32, tag="hT")
                    for c in range(4):
                        nc.tensor.matmul(hps[:], w1t[:, c, :], xT[:, c, :],
                                         start=(c == 0), stop=(c == 3))
                    hT = work.tile([P, P], F32, tag="hT_sb")
                    nc.scalar.activation(hT[:], hps[:], AF.Relu, bias=b1t[:])

                    # pT = W2^T @ hT + b2
                    pps = psA1.tile([P, P], F32, tag="pT")
                    nc.tensor.matmul(pps[:], w2t[:], hT[:])
                    pT = work.tile([P, P], F32, tag="pT_sb")
                    nc.scalar.activation(pT[:], pps[:], AF.Identity, bias=b2t[:])

                    # row sumsq via ones-matmul (partition-axis reduction)
                    sqp = work.tile([P, P], F32, tag="sqp")
                    nc.vector.tensor_tensor(out=sqp[:], in0=pT[:], in1=pT[:],
                                            op=OP.mult)
                    nsq = psA1.tile([1, P], F32, tag="nsq")
                    nc.tensor.matmul(nsq[:], ones_col[:], sqp[:])

                    # norm with one Newton step on sqrt, then clamp+recip
                    n0 = small.tile([1, P], F32, tag="n0")
                    nc.scalar.activation(n0[:], nsq[:], AF.Sqrt, bias=zbias1[:])
                    t0 = small.tile([1, P], F32, tag="t0")
                    nc.vector.reciprocal(t0[:], n0[:])
                    th = small.tile([1, P], F32, tag="th")
                    nc.vector.tensor_tensor(out=th[:], in0=t0[:], in1=nsq[:],
                                            op=OP.mult)
                    th2 = small.tile([1, P], F32, tag="th2")
                    nc.vector.tensor_tensor(out=th2[:], in0=th[:], in1=n0[:],
                                            op=OP.add)
                    ncl = small.tile([1, P], F32, tag="ncl")
                    nc.vector.tensor_scalar(out=ncl[:], in0=th2[:], scalar1=0.5,
                                            scalar2=1e-6, op0=OP.mult, op1=OP.max)
                    rsz = small.tile([1, P], F32, tag="rsz")
                    nc.vector.reciprocal(rsz[:], ncl[:])

                    # broadcast rsz across partitions via K=1 matmul
                    bc = psA1.tile([P, P], F32, tag="bc")
                    nc.tensor.matmul(bc[:], ones_row[:], rsz[:])
                    nc.vector.tensor_tensor(out=zT[:, rs], in0=pT[:], in1=bc[:],
                                            op=OP.mult)

                    # pos = nsq * rsz^2 / T   (diag of sim for these rows)
                    tmp2 = small.tile([1, P], F32, tag="tmp2")
                    nc.vector.tensor_tensor(out=tmp2[:], in0=nsq[:], in1=rsz[:],
                                            op=OP.mult)
                    nc.vector.scalar_tensor_tensor(
                        out=pos_all[:, rs], in0=tmp2[:], scalar=INV_T,
                        in1=rsz[:], op0=OP.mult, op1=OP.mult)

            if phase == "A":
                nc.sync.dma_start(out=out_d[:, :], in_=zT[0:1, 0:1])

            if phase in ("AG", "full"):
                # =============== AllGather z^T across cores =================
                nc.sync.dma_start(out=zT_bounce[:, :], in_=zT[:])
                nc.gpsimd.collective_compute(
                    "AllGather",
                    OP.bypass,
                    ins=[zT_bounce[:, :]],
                    outs=[zall_bounce[:, :]],
                    replica_groups=[list(range(N_CORES))],
                )
                for r in range(N_CORES):
                    nc.sync.dma_start(out=zallT[:, r, :],
                                      in_=zall_bounce[r * P:(r + 1) * P, :])

            if phase == "AG":
                nc.sync.dma_start(out=out_d[:, :], in_=zallT[0:1, 0, 0:1])

            if phase == "full":
                # ======== Phase C: sim row-block + fused exp/rowsum =========
                with tc.tile_pool(name="psC", bufs=2, space="PSUM") as psC:
                    for m in range(NBLK):
                        lhsT = zT[:, m * P:(m + 1) * P]
                        sacc = small.tile([P, 4], F32, tag="sacc")
                        for g in range(4):
                            ps = psC.tile([P, 4, 512], F32, tag="sim")
                            for j in range(4):
                                col = g * 2048 + j * 512
                                r, off = divmod(col, ROWS)
                                nc.tensor.matmul(ps[:, j, :], lhsT,
                                                 zallT[:, r, off:off + 512])
                            sc = expsc.tile([P, 4, 512], F32, tag="expout")
                            nc.scalar.activation(sc[:], ps[:], AF.Exp,
                                                 bias=zbias[:], scale=INV_T,
                                                 accum_out=sacc[:, g:g + 1])
                        S = small.tile([P, 1], F32, tag="S")
                        nc.vector.tensor_reduce(out=S[:], in_=sacc[:],
                                                axis=mybir.AxisListType.X,
                                                op=OP.add)
                        nc.scalar.activation(logS[:, m:m + 1], S[:], AF.Ln,
                                             bias=zbias[:])

                    # final local reduction: out = sum(logS) - sum(pos)
                    possum = small.tile([1, 1], F32, tag="possum")
                    nc.vector.tensor_reduce(out=possum[:], in_=pos_all[:],
                                            axis=mybir.AxisListType.X,
                                            op=OP.add)
                    lps = psC.tile([1, NBLK], F32, tag="sim")
                    nc.tensor.matmul(lps[:], ones_col[:], logS[:])
                    lsum = small.tile([1, 1], F32, tag="lsum")
                    nc.vector.tensor_reduce(out=lsum[:], in_=lps[:],
                                            axis=mybir.AxisListType.X,
                                            op=OP.add)
                    res = small.tile([1, 1], F32, tag="res")
                    nc.vector.tensor_tensor(out=res[:], in0=lsum[:],
                                            in1=possum[:], op=OP.subtract)
                    nc.sync.dma_start(out=out_d[:, :], in_=res[:])

    split_excess_waits(nc)
    return nc


_NC_CACHE = None


def _get_nc():
    global _NC_CACHE
    if _NC_CACHE is None:
        _NC_CACHE = build_nc()
    return _NC_CACHE


def run_spmd(inputs, trace=False, **kw):
    feats = np.ascontiguousarray(inputs["features"], dtype=np.float32)
    n1 = np.ascontiguousarray(inputs["noise1"], dtype=np.float32)
    n2 = np.ascontiguousarray(inputs["noise2"], dtype=np.float32)
    w1 = np.ascontiguousarray(inputs["W1"], dtype=np.float32)
    b1 = np.ascontiguousarray(inputs["b1"], dtype=np.float32).reshape(D_PROJ, 1)
    w2 = np.ascontiguousarray(inputs["W2"], dtype=np.float32)
    b2 = np.ascontiguousarray(inputs["b2"], dtype=np.float32).reshape(D_PROJ, 1)

    in_maps = []
    for r in range(N_CORES):
        sl = slice(r * ROWS, (r + 1) * ROWS)
        in_maps.append({
            "features": feats[sl], "noise1": n1[sl], "noise2": n2[sl],
            "W1": w1, "b1": b1, "W2": w2, "b2": b2,
        })
    nc = _get_nc()
    return run_bass_kernel_spmd(nc, in_maps, core_ids=list(range(N_CORES)),
                                trace=trace, **kw)


def kernel(**inputs) -> np.ndarray:
    out = run_spmd(inputs)
    total = sum(float(out.results[r]["out"][0, 0]) for r in range(N_CORES))
    loss = total / float(N) + float(np.log(np.float32(2.0)))
    return np.array(loss, dtype=np.float32)



# revision 13
# speedup vs baseline: 1.3837x; 1.3837x over previous
"""Distributed Trainium2 (Bass/Tile) kernel for the KPCL contrastive loss.

Math (matches the jax reference):
  x1 = f + sign(f) * normalize(n1, 1e-8) * 0.1
  x2 = x1 + sign(x1) * normalize(n2, 1e-8) * 0.1
     = sign(f) * (|f| + u1/max(10*||u1||,1e-7) + u2/max(10*||u2||,1e-7))
  p  = relu(x2 @ W1 + b1) @ W2 + b2
  z  = p / max(||p||, 1e-6)
  sim = z @ z_all.T / T ;  lse_i = log(sum_j exp(sim_ij)) ; pos_i = sim_ii
  loss = mean(-pos + lse) + log(2)

Sharding: rows (N=8192) split across 8 cores, 1024 rows each. Each core
computes its z block in transposed layout zT [128, 1024] (bf16), and the
zT columns are AllGathered in two 512-column chunks (each [128,512] bf16,
fired as soon as its half of the local rows is done so the collective
overlaps the rest of phase A). Phase C computes the row-block of
sim = zT_m^T @ z_all^T as bf16 128x512 matmuls (4x faster PE than fp32)
with fused exp+row-sum on the activation engine. Per-core output is
[128, 16] (per-partition log-sum-exp values and diag terms); the host
does the final scalar reduction.
"""

import sys

for _p in ("/opt/trn_rl_repo",):
    if _p not in sys.path:
        sys.path.append(_p)

import numpy as np

import concourse.bass as bass
import concourse.tile as tile
from concourse import mybir
from concourse.bass_utils import run_bass_kernel_spmd
from concourse.masks import make_identity

F32 = mybir.dt.float32
BF16 = mybir.dt.bfloat16
U32 = mybir.dt.uint32

N_CORES = 8
N = 8192
ROWS = N // N_CORES          # 1024 rows per core
D_IN = 512
D_PROJ = 128
TEMP = 0.15
P = 128                      # partitions
NBLK = ROWS // P             # 8 row-blocks per core
GB = 4                       # blocks per group (AllGather chunk)
NGRP = NBLK // GB            # 2 groups
INV_T = 1.0 / TEMP

AF = mybir.ActivationFunctionType
OP = mybir.AluOpType


def split_excess_waits(nc: bass.Bass, max_waits: int = 1) -> int:
    """Hoist excess sem waits onto same-engine nop carriers.

    The walrus build in this image rejects instructions carrying more
    than ~2 sync commands ("Too many sync wait commands"), but Tile's
    wait assignment freely emits 2-3 waits per instruction. Splitting
    the waits onto preceding nop instructions on the same engine queue
    is semantically identical (engine program order is preserved).
    """
    nmoved = 0
    for f in nc.m.functions:
        for b in f.blocks:
            il = b.instructions
            i = 0
            while i < len(il):
                inst = il[i]
                si = inst.sync_info
                if si is None or not si.on_wait or len(si.on_wait) <= max_waits:
                    i += 1
                    continue
                eng = inst.engine
                if eng is None:
                    i += 1
                    continue
                waits = list(si.on_wait)
                keep = waits[-max_waits:]
                excess = waits[:-max_waits]
                carriers = []
                for w in excess:
                    nop = nc.engines[eng].nop().ins
                    for f2 in nc.m.functions:
                        for b2 in f2.blocks:
                            try:
                                b2.instructions.remove(nop)
                            except ValueError:
                                pass
                    nop.sync_info = mybir.SyncInfo(on_wait=[w], on_update=[])
                    carriers.append(nop)
                inst.sync_info = mybir.SyncInfo(on_wait=keep,
                                                on_update=list(si.on_update))
                for c in reversed(carriers):
                    il.insert(i, c)
                i += 1 + len(carriers)
                nmoved += len(excess)
    return nmoved


def build_nc() -> bass.Bass:
    nc = bass.Bass("TRN2", target_bir_lowering=False, debug=False,
                   num_devices=N_CORES)

    f_d = nc.dram_tensor("features", [ROWS, D_IN], F32, kind="ExternalInput")
    u1_d = nc.dram_tensor("noise1", [ROWS, D_IN], F32, kind="ExternalInput")
    u2_d = nc.dram_tensor("noise2", [ROWS, D_IN], F32, kind="ExternalInput")
    w1_d = nc.dram_tensor("W1", [D_IN, D_PROJ], F32, kind="ExternalInput")
    b1_d = nc.dram_tensor("b1", [D_PROJ, 1], F32, kind="ExternalInput")
    w2_d = nc.dram_tensor("W2", [D_PROJ, D_PROJ], F32, kind="ExternalInput")
    b2_d = nc.dram_tensor("b2", [D_PROJ, 1], F32, kind="ExternalInput")
    out_d = nc.dram_tensor("out", [P, 2 * NBLK], F32, kind="ExternalOutput")

    # collective bounce buffers per chunk (AG output must be Shared)
    zTb = [nc.dram_tensor(f"zTb{g}", [P, GB * P], BF16) for g in range(NGRP)]
    zallb = [nc.dram_tensor(f"zallb{g}", [N_CORES * P, GB * P], BF16,
                            addr_space="Shared") for g in range(NGRP)]

    with tile.TileContext(nc) as tc:
        with (
            tc.tile_pool(name="singles", bufs=1) as singles,
            tc.tile_pool(name="grp", bufs=2) as grp,
            tc.tile_pool(name="wk", bufs=2) as wk,
            tc.tile_pool(name="sj", bufs=2) as sj,
            tc.tile_pool(name="zr", bufs=2) as zr,
            tc.tile_pool(name="small", bufs=2) as small,
            tc.tile_pool(name="expsc", bufs=2) as expsc,
        ):
            # ---- constants / persistent tiles ----
            w1t = singles.tile([P, 4, P], F32)      # W1 k-chunks (lhsT)
            for c in range(4):
                nc.sync.dma_start(w1t[:, c, :], w1_d[c * P:(c + 1) * P, :])
            w2t = singles.tile([P, P], F32)
            nc.sync.dma_start(w2t[:], w2_d[:, :])
            b1t = singles.tile([P, 1], F32)
            nc.sync.dma_start(b1t[:], b1_d[:, :])
            b2t = singles.tile([P, 1], F32)
            nc.sync.dma_start(b2t[:], b2_d[:, :])

            ident = singles.tile([P, P], F32)
            make_identity(nc, ident[:])
            identb = singles.tile([P, P], BF16)
            nc.any.tensor_copy(identb[:], ident[:])
            zbias = singles.tile([P, 1], F32)
            nc.gpsimd.memset(zbias[:], 0.0)

            s1all = singles.tile([P, NBLK], F32)    # ||u1||^2 per row
            s2all = singles.tile([P, NBLK], F32)
            nsqP = singles.tile([P, NBLK], F32)     # ||p||^2 per row (col layout)
            zT = singles.tile([P, ROWS], BF16)      # z^T for this core
            zallT = [singles.tile([P, N_CORES, GB * P], BF16,
                                  name=f"zallT{g}", tag=f"zallT{g}")
                     for g in range(NGRP)]
            sacc = singles.tile([P, NBLK, 4], F32)  # partial exp row-sums
            Stot = singles.tile([P, NBLK], F32)
            outb = singles.tile([P, 2 * NBLK], F32)  # [logS | pos]

            # ---- all input DMAs up front (block-major) ----
            ftg, u1g, u2g = [], [], []
            for g in range(NGRP):
                ftg.append(grp.tile([P, GB, D_IN], F32, name=f"ft{g}",
                                    tag="F"))
                u1g.append(grp.tile([P, GB, D_IN], F32, name=f"u1t{g}",
                                    tag="U1"))
                u2g.append(grp.tile([P, GB, D_IN], F32, name=f"u2t{g}",
                                    tag="U2"))
            for g in range(NGRP):
                for mm in range(GB):
                    rs = slice((g * GB + mm) * P, (g * GB + mm + 1) * P)
                    nc.sync.dma_start(u1g[g][:, mm, :], u1_d[rs, :])
                    nc.sync.dma_start(u2g[g][:, mm, :], u2_d[rs, :])
                    nc.sync.dma_start(ftg[g][:, mm, :], f_d[rs, :])

            # =========== Phase A: augment + projection + normalize ==========
            with (
                tc.tile_pool(name="psT", bufs=2, space="PSUM") as psT,
                tc.tile_pool(name="psH", bufs=2, space="PSUM") as psH,
                tc.tile_pool(name="psQ", bufs=3, space="PSUM") as psQ,
                tc.tile_pool(name="psZ", bufs=1, space="PSUM") as psZ,
            ):
                for g in range(NGRP):
                    g4 = slice(g * GB, (g + 1) * GB)
                    # --- stage 1: row sums of squares + |f| ---
                    for mm in range(GB):
                        m = g * GB + mm
                        junk = sj.tile([P, D_IN], BF16, tag="sqj")
                        nc.scalar.activation(junk[:], u1g[g][:, mm, :],
                                             AF.Square, bias=zbias[:],
                                             accum_out=s1all[:, m:m + 1])
                    abst = []
                    for mm in range(GB):
                        ab = wk.tile([P, D_IN], F32, tag="absf", bufs=5)
                        nc.scalar.activation(ab[:], ftg[g][:, mm, :], AF.Abs,
                                             bias=zbias[:])
                        abst.append(ab)
                    sgnt = []
                    for mm in range(GB):
                        sg = wk.tile([P, D_IN], F32, tag="sgn", bufs=5)
                        nc.scalar.activation(sg[:], ftg[g][:, mm, :], AF.Sign,
                                             bias=zbias[:])
                        sgnt.append(sg)
                    for mm in range(GB):
                        m = g * GB + mm
                        junk = sj.tile([P, D_IN], BF16, tag="sqj")
                        nc.vector.scalar_tensor_tensor(
                            out=junk[:], in0=u2g[g][:, mm, :], scalar=1.0,
                            in1=u2g[g][:, mm, :], op0=OP.mult, op1=OP.mult,
                            accum_out=s2all[:, m:m + 1])

                    # --- stage 2: noise scales r = 1/max(10*||u||, 1e-7) ---
                    n1gt = small.tile([P, GB], F32, tag="n1g")
                    nc.scalar.activation(n1gt[:], s1all[:, g4], AF.Sqrt,
                                         bias=zbias[:], scale=100.0)
                    n2gt = small.tile([P, GB], F32, tag="n2g")
                    nc.scalar.activation(n2gt[:], s2all[:, g4], AF.Sqrt,
                                         bias=zbias[:], scale=100.0)
                    n1c = small.tile([P, GB], F32, tag="n1c")
                    nc.vector.tensor_scalar(out=n1c[:], in0=n1gt[:],
                                            scalar1=1e-7, scalar2=None,
                                            op0=OP.max)
                    r1g = small.tile([P, GB], F32, tag="r1g")
                    nc.vector.reciprocal(r1g[:], n1c[:])
                    n2c = small.tile([P, GB], F32, tag="n2c")
                    nc.vector.tensor_scalar(out=n2c[:], in0=n2gt[:],
                                            scalar1=1e-7, scalar2=None,
                                            op0=OP.max)
                    r2g = small.tile([P, GB], F32, tag="r2g")
                    nc.vector.reciprocal(r2g[:], n2c[:])

                    # --- stage 3: x2 = (|f| + u1*r1 + u2*r2) | signbit(f) ---
                    xT = grp.tile([P, 4, GB * P], F32, tag="xT")
                    for mm in range(GB):
                        m = g * GB + mm
                        bt = wk.tile([P, D_IN], F32, tag="bt")
                        nc.vector.scalar_tensor_tensor(
                            out=bt[:], in0=u1g[g][:, mm, :],
                            scalar=r1g[:, mm:mm + 1], in1=abst[mm][:],
                            op0=OP.mult, op1=OP.add)
                        at = wk.tile([P, D_IN], F32, tag="at")
                        nc.vector.scalar_tensor_tensor(
                            out=at[:], in0=u2g[g][:, mm, :],
                            scalar=r2g[:, mm:mm + 1], in1=bt[:],
                            op0=OP.mult, op1=OP.add)
                        x2 = wk.tile([P, D_IN], F32, tag="x2")
                        nc.vector.tensor_tensor(out=x2[:], in0=at[:],
                                                in1=sgnt[mm][:], op=OP.mult)
                        for c in range(4):
                            tp = psT.tile([P, P], F32, tag="tp")
                            nc.tensor.transpose(tp[:],
                                                x2[:, c * P:(c + 1) * P],
                                                ident[:])
                            nc.any.tensor_copy(
                                xT[:, c, mm * P:(mm + 1) * P], tp[:])

                    # --- stage 4: projection for the group (free dim 512) ---
                    hps = psH.tile([P, GB * P], F32, tag="hp")
                    for c in range(4):
                        nc.tensor.matmul(hps[:], w1t[:, c, :], xT[:, c, :],
                                         start=(c == 0), stop=(c == 3))
                    hT = grp.tile([P, GB * P], F32, tag="hT")
                    nc.scalar.activation(hT[:], hps[:], AF.Relu, bias=b1t[:])
                    pps = psH.tile([P, GB * P], F32, tag="hp")
                    nc.tensor.matmul(pps[:], w2t[:], hT[:])
                    pT = grp.tile([P, GB * P], F32, tag="pT")
                    nc.scalar.activation(pT[:], pps[:], AF.Identity,
                                         bias=b2t[:])

                    # --- stage 5: per-row ||p||^2 via transpose + square ---
                    prows = []
                    for mm in range(GB):
                        m = g * GB + mm
                        tpp = psQ.tile([P, P], F32, tag="tpp")
                        nc.tensor.transpose(tpp[:], pT[:, mm * P:(mm + 1) * P],
                                            ident[:])
                        prow = zr.tile([P, P], F32, name=f"prow{g}{mm}",
                                       tag="prow", bufs=5)
                        nc.any.tensor_copy(prow[:], tpp[:])
                        njunk = sj.tile([P, P], BF16, tag="nj")
                        nc.scalar.activation(njunk[:], tpp[:], AF.Square,
                                             bias=zbias[:],
                                             accum_out=nsqP[:, m:m + 1])
                        prows.append(prow)

                    # --- stage 6: rsz = 1/||p|| with one Newton step; pos ---
                    n0 = small.tile([P, GB], F32, tag="n0")
                    nc.scalar.activation(n0[:], nsqP[:, g4], AF.Sqrt,
                                         bias=zbias[:])
                    rsz0 = small.tile([P, GB], F32, tag="rsz0")
                    nc.vector.reciprocal(rsz0[:], n0[:])
                    t1 = small.tile([P, GB], F32, tag="t1")
                    nc.vector.tensor_tensor(out=t1[:], in0=rsz0[:],
                                            in1=rsz0[:], op=OP.mult)
                    t2 = small.tile([P, GB], F32, tag="t2")
                    nc.vector.tensor_tensor(out=t2[:], in0=t1[:],
                                            in1=nsqP[:, g4], op=OP.mult)
                    t3 = small.tile([P, GB], F32, tag="t3")
                    nc.vector.tensor_scalar(out=t3[:], in0=t2[:], scalar1=-0.5,
                                            scalar2=1.5, op0=OP.mult,
                                            op1=OP.add)
                    rsz = small.tile([P, GB], F32, tag="rsz")
                    nc.vector.tensor_tensor(out=rsz[:], in0=rsz0[:],
                                            in1=t3[:], op=OP.mult)
                    av = small.tile([P, GB], F32, tag="av")
                    nc.vector.tensor_tensor(out=av[:], in0=nsqP[:, g4],
                                            in1=rsz[:], op=OP.mult)
                    # pos = nsq * rsz^2 / T  (diag of sim, fp32 path)
                    nc.vector.scalar_tensor_tensor(
                        out=outb[:, NBLK + g * GB:NBLK + (g + 1) * GB],
                        in0=av[:], scalar=INV_T, in1=rsz[:],
                        op0=OP.mult, op1=OP.mult)

                    # --- stage 7: z rows = p * rsz; transpose into zT bf16 ---
                    for mm in range(GB):
                        m = g * GB + mm
                        zrows = zr.tile([P, P], BF16, tag="zrows")
                        nc.vector.tensor_scalar(out=zrows[:],
                                                in0=prows[mm][:],
                                                scalar1=rsz[:, mm:mm + 1],
                                                scalar2=None, op0=OP.mult)
                        ztp = psZ.tile([P, P], BF16, tag="ztp")
                        nc.tensor.transpose(ztp[:], zrows[:], identb[:])
                        nc.any.tensor_copy(zT[:, m * P:(m + 1) * P], ztp[:])

                    # --- stage 8: ship this chunk of zT; AllGather it ---
                    cs = slice(g * GB * P, (g + 1) * GB * P)
                    nc.sync.dma_start(out=zTb[g][:, :], in_=zT[:, cs])
                    nc.gpsimd.collective_compute(
                        "AllGather",
                        OP.bypass,
                        ins=[zTb[g][:, :]],
                        outs=[zallb[g][:, :]],
                        replica_groups=[list(range(N_CORES))],
                    )

            # ---- land the gathered chunks in SBUF ----
            for g in range(NGRP):
                for r in range(N_CORES):
                    nc.sync.dma_start(out=zallT[g][:, r, :],
                                      in_=zallb[g][r * P:(r + 1) * P, :])

            # ======== Phase C: sim row-blocks + fused exp/rowsum =========
            with tc.tile_pool(name="psC", bufs=2, space="PSUM") as psC:
                for g in range(NGRP):
                    for m in range(NBLK):
                        lhsT = zT[:, m * P:(m + 1) * P]
                        for h in range(2):
                            ps = psC.tile([P, 4 * 512], F32, tag="ps")
                            for j in range(4):
                                nc.tensor.matmul(
                                    ps[:, j * 512:(j + 1) * 512], lhsT,
                                    zallT[g][:, h * 4 + j, :])
                            eo = expsc.tile([P, 4 * 512], BF16, tag="eo")
                            k = 2 * g + h
                            nc.scalar.activation(
                                eo[:], ps[:], AF.Exp, bias=zbias[:],
                                scale=INV_T,
                                accum_out=sacc[:, m, k:k + 1])

                # ---- final: logS per row; host does the scalar reduce ----
                for m in range(NBLK):
                    nc.vector.tensor_reduce(out=Stot[:, m:m + 1],
                                            in_=sacc[:, m, :],
                                            axis=mybir.AxisListType.X,
                                            op=OP.add)
                nc.scalar.activation(outb[:, 0:NBLK], Stot[:], AF.Ln,
                                     bias=zbias[:])
                nc.sync.dma_start(out=out_d[:, :], in_=outb[:])

    split_excess_waits(nc)
    return nc


_NC_CACHE = None


def _get_nc():
    global _NC_CACHE
    if _NC_CACHE is None:
        _NC_CACHE = build_nc()
    return _NC_CACHE


def finalize_outputs(core_outs) -> np.ndarray:
    """core_outs: list of per-core dicts with 'out' [P, 2*NBLK] f32."""
    total = 0.0
    for arr in core_outs:
        a = np.asarray(arr, dtype=np.float64)
        total += a[:, :NBLK].sum() - a[:, NBLK:].sum()
    loss = total / float(N) + float(np.log(np.float32(2.0)))
    return np.array(loss, dtype=np.float32)


def run_spmd(inputs, trace=False, **kw):
    feats = np.ascontiguousarray(inputs["features"], dtype=np.float32)
    n1 = np.ascontiguousarray(inputs["noise1"], dtype=np.float32)
    n2 = np.ascontiguousarray(inputs["noise2"], dtype=np.float32)
    w1 = np.ascontiguousarray(inputs["W1"], dtype=np.float32)
    b1 = np.ascontiguousarray(inputs["b1"], dtype=np.float32).reshape(D_PROJ, 1)
    w2 = np.ascontiguousarray(inputs["W2"], dtype=np.float32)
    b2 = np.ascontiguousarray(inputs["b2"], dtype=np.float32).reshape(D_PROJ, 1)

    in_maps = []
    for r in range(N_CORES):
        sl = slice(r * ROWS, (r + 1) * ROWS)
        in_maps.append({
            "features": feats[sl], "noise1": n1[sl], "noise2": n2[sl],
            "W1": w1, "b1": b1, "W2": w2, "b2": b2,
        })
    nc = _get_nc()
    return run_bass_kernel_spmd(nc, in_maps, core_ids=list(range(N_CORES)),
                                trace=trace, **kw)


def kernel(**inputs) -> np.ndarray:
    out = run_spmd(inputs)
    return finalize_outputs([out.results[r]["out"] for r in range(N_CORES)])


# revision 23
# speedup vs baseline: 1.6427x; 1.1871x over previous
"""Distributed Trainium2 (Bass/Tile) kernel for the KPCL contrastive loss.

Math (matches the jax reference):
  x1 = f + sign(f) * normalize(n1, 1e-8) * 0.1
  x2 = x1 + sign(x1) * normalize(n2, 1e-8) * 0.1
     = sign(f) * (|f| + u1/max(10*||u1||,1e-7) + u2/max(10*||u2||,1e-7))
  p  = relu(x2 @ W1 + b1) @ W2 + b2
  z  = p / max(||p||, 1e-6)
  sim = z @ z_all.T / T ;  lse_i = log(sum_j exp(sim_ij)) ; pos_i = sim_ii
  loss = mean(-pos + lse) + log(2)

Sharding: rows (N=8192) split across 8 cores, 1024 rows each. Each core
computes its z block in transposed layout zT [128, 8, 128] (bf16), and
the zT columns are AllGathered in two 512-column chunks (each [128,512]
bf16, fired as soon as its half of the local rows is done so the
collective overlaps the rest of phase A). A tiny dummy AllGather is
issued first so the one-time collectives bootstrap barrier runs
concurrently with phase A instead of gating the real data transfers.
Phase C computes the row-block of sim = zT_m^T @ z_all^T as bf16
128x512 matmuls with fused exp+row-sum on the activation engine.
Per-core output is [128, 16] (per-partition log-sum-exp values and diag
terms); the host does the final scalar reduction.

Engine split in phase A: Act does squares/abs/sign/sqrt/relu, DVE does
the augment adds + norms, Pool (gpsimd) does the sign-multiply and all
PSUM->SBUF copies, PE does transposes + the (bf16) projection matmuls.
"""

import sys

for _p in ("/opt/trn_rl_repo",):
    if _p not in sys.path:
        sys.path.append(_p)

import numpy as np

import concourse.bass as bass
import concourse.tile as tile
from concourse import mybir
from concourse.bass_utils import run_bass_kernel_spmd
from concourse.masks import make_identity

F32 = mybir.dt.float32
BF16 = mybir.dt.bfloat16
U32 = mybir.dt.uint32

N_CORES = 8
N = 8192
ROWS = N // N_CORES          # 1024 rows per core
D_IN = 512
D_PROJ = 128
TEMP = 0.15
P = 128                      # partitions
NBLK = ROWS // P             # 8 row-blocks per core
GB = 4                       # blocks per group (AllGather chunk)
NGRP = NBLK // GB            # 2 groups
INV_T = 1.0 / TEMP

AF = mybir.ActivationFunctionType
OP = mybir.AluOpType


def split_excess_waits(nc: bass.Bass, max_waits: int = 1) -> int:
    """Hoist excess sem waits onto same-engine nop carriers.

    The walrus build in this image rejects instructions carrying more
    than ~2 sync commands ("Too many sync wait commands"), but Tile's
    wait assignment freely emits 2-3 waits per instruction. Splitting
    the waits onto preceding nop instructions on the same engine queue
    is semantically identical (engine program order is preserved).
    """
    nmoved = 0
    for f in nc.m.functions:
        for b in f.blocks:
            il = b.instructions
            i = 0
            while i < len(il):
                inst = il[i]
                si = inst.sync_info
                if si is None or not si.on_wait or len(si.on_wait) <= max_waits:
                    i += 1
                    continue
                eng = inst.engine
                if eng is None:
                    i += 1
                    continue
                waits = list(si.on_wait)
                keep = waits[-max_waits:]
                excess = waits[:-max_waits]
                carriers = []
                for w in excess:
                    nop = nc.engines[eng].nop().ins
                    for f2 in nc.m.functions:
                        for b2 in f2.blocks:
                            try:
                                b2.instructions.remove(nop)
                            except ValueError:
                                pass
                    nop.sync_info = mybir.SyncInfo(on_wait=[w], on_update=[])
                    carriers.append(nop)
                inst.sync_info = mybir.SyncInfo(on_wait=keep,
                                                on_update=list(si.on_update))
                for c in reversed(carriers):
                    il.insert(i, c)
                i += 1 + len(carriers)
                nmoved += len(excess)
    return nmoved


def build_nc() -> bass.Bass:
    nc = bass.Bass("TRN2", target_bir_lowering=False, debug=False,
                   num_devices=N_CORES)

    f_d = nc.dram_tensor("features", [ROWS, D_IN], F32, kind="ExternalInput")
    u1_d = nc.dram_tensor("noise1", [ROWS, D_IN], F32, kind="ExternalInput")
    u2_d = nc.dram_tensor("noise2", [ROWS, D_IN], F32, kind="ExternalInput")
    w1_d = nc.dram_tensor("W1", [D_IN, D_PROJ], F32, kind="ExternalInput")
    b1_d = nc.dram_tensor("b1", [D_PROJ, 1], F32, kind="ExternalInput")
    w2_d = nc.dram_tensor("W2", [D_PROJ, D_PROJ], F32, kind="ExternalInput")
    b2_d = nc.dram_tensor("b2", [D_PROJ, 1], F32, kind="ExternalInput")
    out_d = nc.dram_tensor("out", [P, 2 * NBLK], F32, kind="ExternalOutput")

    # dummy collective to pull the one-time bootstrap barrier early
    # (gathers a tiny zero scratch tensor; the result is unused)
    dmy_in = nc.dram_tensor("dmy_in", [P, 1], F32)
    dmy_out = nc.dram_tensor("dmy_out", [N_CORES * P, 1], F32,
                             addr_space="Shared")

    # collective bounce buffers per chunk (AG output must be Shared)
    zTb = [nc.dram_tensor(f"zTb{g}", [P, GB, P], BF16) for g in range(NGRP)]
    zallb = [nc.dram_tensor(f"zallb{g}", [N_CORES * P, GB * P], BF16,
                            addr_space="Shared") for g in range(NGRP)]

    with tile.TileContext(nc) as tc:
        with (
            tc.tile_pool(name="singles", bufs=1) as singles,
            tc.tile_pool(name="grp", bufs=2) as grp,
            tc.tile_pool(name="wk", bufs=2) as wk,
            tc.tile_pool(name="sj", bufs=2) as sj,
            tc.tile_pool(name="zr", bufs=2) as zr,
            tc.tile_pool(name="small", bufs=2) as small,
            tc.tile_pool(name="expsc", bufs=2) as expsc,
        ):
            # fire the dummy collective first: its completion is unused,
            # it only exists to absorb the bootstrap barrier early.
            zbias = singles.tile([P, 1], F32)
            nc.gpsimd.memset(zbias[:], 0.0)
            nc.sync.dma_start(out=dmy_in[:, :], in_=zbias[:])
            nc.gpsimd.collective_compute(
                "AllGather", OP.bypass, ins=[dmy_in[:, :]],
                outs=[dmy_out[:, :]],
                replica_groups=[list(range(N_CORES))],
            )

            # ---- constants / persistent tiles ----
            w1f = singles.tile([P, 4, P], F32)
            for c in range(4):
                nc.sync.dma_start(w1f[:, c, :], w1_d[c * P:(c + 1) * P, :])
            w2f = singles.tile([P, P], F32)
            nc.sync.dma_start(w2f[:], w2_d[:, :])
            b1t = singles.tile([P, 1], F32)
            nc.sync.dma_start(b1t[:], b1_d[:, :])
            b2t = singles.tile([P, 1], F32)
            nc.sync.dma_start(b2t[:], b2_d[:, :])
            w1b = singles.tile([P, 4, P], BF16)
            nc.gpsimd.tensor_copy(w1b[:], w1f[:])
            w2b = singles.tile([P, P], BF16)
            nc.gpsimd.tensor_copy(w2b[:], w2f[:])

            ident = singles.tile([P, P], F32)
            make_identity(nc, ident[:])
            identb = singles.tile([P, P], BF16)
            nc.gpsimd.tensor_copy(identb[:], ident[:])

            s1all = singles.tile([P, NBLK], F32)    # ||u1||^2 per row
            s2all = singles.tile([P, NBLK], F32)
            nsqP = singles.tile([P, NBLK], F32)     # ||p||^2 per row (col layout)
            zT = singles.tile([P, NBLK, P], BF16)   # z^T for this core
            zallT = [singles.tile([P, N_CORES, GB * P], BF16,
                                  name=f"zallT{g}", tag=f"zallT{g}")
                     for g in range(NGRP)]
            sacc = singles.tile([P, NBLK, 4], F32)  # partial exp row-sums
            Stot = singles.tile([P, NBLK], F32)
            outb = singles.tile([P, 2 * NBLK], F32)  # [logS | pos]

            # ---- all input DMAs up front (block-major) ----
            ftg, u1g, u2g = [], [], []
            for g in range(NGRP):
                ftg.append(grp.tile([P, GB, D_IN], F32, name=f"ft{g}",
                                    tag="F"))
                u1g.append(grp.tile([P, GB, D_IN], F32, name=f"u1t{g}",
                                    tag="U1"))
                u2g.append(grp.tile([P, GB, D_IN], F32, name=f"u2t{g}",
                                    tag="U2"))
            for g in range(NGRP):
                for mm in range(GB):
                    rs = slice((g * GB + mm) * P, (g * GB + mm + 1) * P)
                    nc.sync.dma_start(u1g[g][:, mm, :], u1_d[rs, :])
                    nc.sync.dma_start(ftg[g][:, mm, :], f_d[rs, :])
                    nc.sync.dma_start(u2g[g][:, mm, :], u2_d[rs, :])

            # =========== Phase A: augment + projection + normalize ==========
            with (
                tc.tile_pool(name="psT", bufs=2, space="PSUM") as psT,
                tc.tile_pool(name="psH", bufs=2, space="PSUM") as psH,
                tc.tile_pool(name="psQ", bufs=2, space="PSUM") as psQ,
                tc.tile_pool(name="psZ", bufs=2, space="PSUM") as psZ,
            ):
                for g in range(NGRP):
                    g4 = slice(g * GB, (g + 1) * GB)
                    # --- stage 1: row sums of squares, |f|, sign(f) ---
                    for mm in range(GB):
                        m = g * GB + mm
                        junk = sj.tile([P, D_IN], BF16, tag="sqj")
                        nc.scalar.activation(junk[:], u1g[g][:, mm, :],
                                             AF.Square, bias=zbias[:],
                                             accum_out=s1all[:, m:m + 1])
                    sgnt = []
                    for mm in range(GB):
                        sg = wk.tile([P, D_IN], F32, tag="sgn", bufs=5)
                        nc.scalar.activation(sg[:], ftg[g][:, mm, :], AF.Sign,
                                             bias=zbias[:])
                        sgnt.append(sg)
                    for mm in range(GB):
                        m = g * GB + mm
                        junk = sj.tile([P, D_IN], BF16, tag="sqj")
                        nc.vector.scalar_tensor_tensor(
                            out=junk[:], in0=u2g[g][:, mm, :], scalar=1.0,
                            in1=u2g[g][:, mm, :], op0=OP.mult, op1=OP.mult,
                            accum_out=s2all[:, m:m + 1])

                    # --- stage 2: noise scales r = 1/max(10*||u||, 1e-7) ---
                    n1gt = small.tile([P, GB], F32, tag="n1g")
                    nc.scalar.activation(n1gt[:], s1all[:, g4], AF.Sqrt,
                                         bias=zbias[:], scale=100.0)
                    n2gt = small.tile([P, GB], F32, tag="n2g")
                    nc.scalar.activation(n2gt[:], s2all[:, g4], AF.Sqrt,
                                         bias=zbias[:], scale=100.0)
                    n1c = small.tile([P, GB], F32, tag="n1c")
                    nc.vector.tensor_scalar(out=n1c[:], in0=n1gt[:],
                                            scalar1=1e-7, scalar2=None,
                                            op0=OP.max)
                    r1g = small.tile([P, GB], F32, tag="r1g")
                    nc.vector.reciprocal(r1g[:], n1c[:])
                    n2c = small.tile([P, GB], F32, tag="n2c")
                    nc.vector.tensor_scalar(out=n2c[:], in0=n2gt[:],
                                            scalar1=1e-7, scalar2=None,
                                            op0=OP.max)
                    r2g = small.tile([P, GB], F32, tag="r2g")
                    nc.vector.reciprocal(r2g[:], n2c[:])

                    # --- stage 3: x2 = f + sign(f)*(u1*r1 + u2*r2) ---
                    xTb = grp.tile([P, 4, GB * P], BF16, tag="xT")
                    for mm in range(GB):
                        m = g * GB + mm
                        d1 = wk.tile([P, D_IN], F32, tag="d1")
                        nc.vector.tensor_scalar(out=d1[:],
                                                in0=u1g[g][:, mm, :],
                                                scalar1=r1g[:, mm:mm + 1],
                                                scalar2=None, op0=OP.mult)
                        dt = wk.tile([P, D_IN], F32, tag="dt")
                        nc.vector.scalar_tensor_tensor(
                            out=dt[:], in0=u2g[g][:, mm, :],
                            scalar=r2g[:, mm:mm + 1], in1=d1[:],
                            op0=OP.mult, op1=OP.add)
                        sd = wk.tile([P, D_IN], F32, tag="sd")
                        nc.gpsimd.tensor_tensor(out=sd[:], in0=dt[:],
                                                in1=sgnt[mm][:], op=OP.mult)
                        x2 = wk.tile([P, D_IN], F32, tag="x2")
                        nc.gpsimd.tensor_tensor(out=x2[:], in0=sd[:],
                                                in1=ftg[g][:, mm, :],
                                                op=OP.add)
                        xps = psT.tile([P, 4, P], F32, tag="xps")
                        for c in range(4):
                            nc.tensor.transpose(xps[:, c, :],
                                                x2[:, c * P:(c + 1) * P],
                                                ident[:])
                        if mm % 2 == 0:
                            nc.scalar.activation(
                                xTb[:, :, mm * P:(mm + 1) * P], xps[:],
                                AF.Copy)
                        else:
                            nc.vector.tensor_copy(
                                xTb[:, :, mm * P:(mm + 1) * P], xps[:])

                    # --- stage 4: projection for the group (free dim 512) ---
                    hps = psH.tile([P, GB * P], F32, tag="hp")
                    for c in range(4):
                        nc.tensor.matmul(hps[:], w1b[:, c, :], xTb[:, c, :],
                                         start=(c == 0), stop=(c == 3))
                    hT = grp.tile([P, GB * P], BF16, tag="hT")
                    nc.scalar.activation(hT[:], hps[:], AF.Relu, bias=b1t[:])
                    pps = psH.tile([P, GB * P], F32, tag="hp")
                    nc.tensor.matmul(pps[:], w2b[:], hT[:])
                    pT = grp.tile([P, GB * P], F32, tag="pT")
                    nc.scalar.activation(pT[:], pps[:], AF.Identity,
                                         bias=b2t[:])

                    # --- stage 5: p rows + per-row ||p||^2 ---
                    tppg = psQ.tile([P, GB, P], F32, tag="tppg")
                    for mm in range(GB):
                        m = g * GB + mm
                        nc.tensor.transpose(tppg[:, mm, :],
                                            pT[:, mm * P:(mm + 1) * P],
                                            ident[:])
                        njunk = sj.tile([P, P], BF16, tag="nj")
                        nc.scalar.activation(njunk[:], tppg[:, mm, :],
                                             AF.Square, bias=zbias[:],
                                             accum_out=nsqP[:, m:m + 1])

                    # --- stage 6: rsz = 1/||p|| with one Newton step; pos ---
                    n0 = small.tile([P, GB], F32, tag="n0")
                    nc.scalar.activation(n0[:], nsqP[:, g4], AF.Sqrt,
                                         bias=zbias[:])
                    rsz0 = small.tile([P, GB], F32, tag="rsz0")
                    nc.vector.reciprocal(rsz0[:], n0[:])
                    t1 = small.tile([P, GB], F32, tag="t1")
                    nc.vector.tensor_tensor(out=t1[:], in0=rsz0[:],
                                            in1=rsz0[:], op=OP.mult)
                    t2 = small.tile([P, GB], F32, tag="t2")
                    nc.vector.tensor_tensor(out=t2[:], in0=t1[:],
                                            in1=nsqP[:, g4], op=OP.mult)
                    t3 = small.tile([P, GB], F32, tag="t3")
                    nc.vector.tensor_scalar(out=t3[:], in0=t2[:], scalar1=-0.5,
                                            scalar2=1.5, op0=OP.mult,
                                            op1=OP.add)
                    rsz = small.tile([P, GB], F32, tag="rsz")
                    nc.vector.tensor_tensor(out=rsz[:], in0=rsz0[:],
                                            in1=t3[:], op=OP.mult)
                    av = small.tile([P, GB], F32, tag="av")
                    nc.vector.tensor_tensor(out=av[:], in0=nsqP[:, g4],
                                            in1=rsz[:], op=OP.mult)
                    # pos = nsq * rsz^2 / T  (diag of sim, fp32 path)
                    nc.vector.scalar_tensor_tensor(
                        out=outb[:, NBLK + g * GB:NBLK + (g + 1) * GB],
                        in0=av[:], scalar=INV_T, in1=rsz[:],
                        op0=OP.mult, op1=OP.mult)

                    # --- stage 7: z rows = p * rsz; transpose into zT bf16 ---
                    zrg = zr.tile([P, GB, P], BF16, tag="zrg")
                    ztpg = psZ.tile([P, GB, P], BF16, tag="ztpg")
                    for mm in range(GB):
                        nc.vector.tensor_scalar(out=zrg[:, mm, :],
                                                in0=tppg[:, mm, :],
                                                scalar1=rsz[:, mm:mm + 1],
                                                scalar2=None, op0=OP.mult)
                        nc.tensor.transpose(ztpg[:, mm, :], zrg[:, mm, :],
                                            identb[:])
                    nc.vector.tensor_copy(zT[:, g4, :], ztpg[:])

                    # --- stage 8: ship this chunk of zT; AllGather it ---
                    nc.sync.dma_start(out=zTb[g][:, :, :], in_=zT[:, g4, :])
                    nc.gpsimd.collective_compute(
                        "AllGather",
                        OP.bypass,
                        ins=[zTb[g][:, :, :]],
                        outs=[zallb[g][:, :]],
                        replica_groups=[list(range(N_CORES))],
                    )

            # ---- land the gathered chunks in SBUF ----
            for g in range(NGRP):
                for r in range(N_CORES):
                    nc.sync.dma_start(out=zallT[g][:, r, :],
                                      in_=zallb[g][r * P:(r + 1) * P, :])

            # ======== Phase C: sim row-blocks + fused exp/rowsum =========
            with tc.tile_pool(name="psC", bufs=2, space="PSUM") as psC:
                for g in range(NGRP):
                    for m in range(NBLK):
                        lhsT = zT[:, m, :]
                        for h in range(2):
                            ps = psC.tile([P, 4 * 512], F32, tag="ps")
                            for j in range(4):
                                nc.tensor.matmul(
                                    ps[:, j * 512:(j + 1) * 512], lhsT,
                                    zallT[g][:, h * 4 + j, :])
                            eo = expsc.tile([P, 4 * 512], BF16, tag="eo")
                            k = 2 * g + h
                            nc.scalar.activation(
                                eo[:], ps[:], AF.Exp, bias=zbias[:],
                                scale=INV_T,
                                accum_out=sacc[:, m, k:k + 1])

                # ---- final: logS per row; host does the scalar reduce ----
                for m in range(NBLK):
                    nc.vector.tensor_reduce(out=Stot[:, m:m + 1],
                                            in_=sacc[:, m, :],
                                            axis=mybir.AxisListType.X,
                                            op=OP.add)
                nc.scalar.activation(outb[:, 0:NBLK], Stot[:], AF.Ln,
                                     bias=zbias[:])
                nc.sync.dma_start(out=out_d[:, :], in_=outb[:])

    split_excess_waits(nc)
    return nc


_NC_CACHE = None


def _get_nc():
    global _NC_CACHE
    if _NC_CACHE is None:
        _NC_CACHE = build_nc()
    return _NC_CACHE


def finalize_outputs(core_outs) -> np.ndarray:
    """core_outs: list of per-core arrays 'out' [P, 2*NBLK] f32."""
    total = 0.0
    for arr in core_outs:
        a = np.asarray(arr, dtype=np.float64)
        total += a[:, :NBLK].sum() - a[:, NBLK:].sum()
    loss = total / float(N) + float(np.log(np.float32(2.0)))
    return np.array(loss, dtype=np.float32)


def run_spmd(inputs, trace=False, **kw):
    feats = np.ascontiguousarray(inputs["features"], dtype=np.float32)
    n1 = np.ascontiguousarray(inputs["noise1"], dtype=np.float32)
    n2 = np.ascontiguousarray(inputs["noise2"], dtype=np.float32)
    w1 = np.ascontiguousarray(inputs["W1"], dtype=np.float32)
    b1 = np.ascontiguousarray(inputs["b1"], dtype=np.float32).reshape(D_PROJ, 1)
    w2 = np.ascontiguousarray(inputs["W2"], dtype=np.float32)
    b2 = np.ascontiguousarray(inputs["b2"], dtype=np.float32).reshape(D_PROJ, 1)

    in_maps = []
    for r in range(N_CORES):
        sl = slice(r * ROWS, (r + 1) * ROWS)
        in_maps.append({
            "features": feats[sl], "noise1": n1[sl], "noise2": n2[sl],
            "W1": w1, "b1": b1, "W2": w2, "b2": b2,
        })
    nc = _get_nc()
    return run_bass_kernel_spmd(nc, in_maps, core_ids=list(range(N_CORES)),
                                trace=trace, **kw)


def kernel(**inputs) -> np.ndarray:
    out = run_spmd(inputs)
    return finalize_outputs([out.results[r]["out"] for r in range(N_CORES)])
